# revision 36
# baseline (speedup 1.0000x reference)
"""Trainium2 Bass kernel for nn_CFTAOBlock2D (v17, ~169us vs 214us v3).

Sharding: pure data-parallel over (batch b, channel-half) -> 8 cores.

Key structure:
  - all 8 depthwise 3x3 taps run on PE as 4 fp8e4m3 DoubleRow
    pair-matmuls against a pitch-257 (1 zero pad col per row) fp8 copy
    of x (xq8). The pad column zeroes edge-column reads -> no boundary
    corrections and no misaligned DVE tap ops. DoubleRow rhs lanes are
    two offset slices of one buffer via a hand-built 4D AP.
  - local 1x1 + mlp1 stay fp16 (fp8 there costs ~1.8e-2 rel err; taps
    in fp8 cost only ~3e-4), streamed per-hc from channel-major xbh.
  - per-quarter mlp1 psum tiles with immediate per-1024 gelus on Act;
    zbuf merge (+szc accum) on DVE, squares on gpsimd (DVE for the
    last chunk), sum-reduce on DVE. Act stays gelu-only in the loop.
  - spectral cp shares the loop's h1 psum pool and Zh stages run from
    the (still free) ZP pool BEFORE part1(0/1) -- keeping the zh tile
    rotation off the Act-gelu dependency chain; transpose staging is
    one fused strided DVE copy per block so the Act drains its T1
    copies early.
  - output is gelu-only fp16 (4096-wide final sweep); the +x residual
    is added on host in f32.
  - known limits: steady loop is Act-chain bound (~5.6us/chunk); the
    head cold-clock window (~27-51us) resists reordering; runs vary
    ~3% with device P0 state (up to +20% right after a hot run).
"""
from contextlib import ExitStack

import numpy as np
import ml_dtypes

import bass_rust
import concourse.bass as bass
import concourse.bacc as bacc
import concourse.tile as tile
from concourse import mybir
from concourse.bass_utils import run_bass_kernel_spmd

F32 = mybir.dt.float32
FP16 = mybir.dt.float16
FP8 = mybir.dt.float8e4
NP8 = ml_dtypes.float8_e4m3
AX = mybir.AluOpType
AF = mybir.ActivationFunctionType
APc = bass_rust.AP

B, C, H, W = 4, 64, 256, 256
M1, M2 = 32, 32
HALF_M = 16
LSEG, RADIAL_K = 4, 4
LOCAL_SCALE, SPATIAL_SCALE, SPEC_SCALE = 0.3, 0.15, 1.0
HW = H * W            # 65536
NQ = 4                # H quarters
FQ = 16384            # dense free per quarter
NHC = 16              # half-chunks of 1024 cols
HC = 1024
OC = 32               # own channels per core
N_CORES = 8

# pitch-257 packed geometry for the tap buffer
WP = 257              # row pitch (256 data + 1 zero pad)
FQP = 64 * WP         # 16448 per quarter
HALO_P = 258
XB_P = FQP + 2 * HALO_P   # 16964

# DoubleRow tap pairs: (d0, d1) flat offsets at pitch 257
TAP_PAIRS = [(-258, 258), (-257, -1), (-256, 256), (1, 257)]


def _tap_dyx(d):
    # d = 257*dy + dx with dy in {-1,0,1}, dx in {-1,0,1}
    for dy in (-1, 0, 1):
        dx = d - 257 * dy
        if -1 <= dx <= 1:
            return dy, dx
    raise ValueError(d)


# ---------------------------------------------------------------- host math
def _softplus(x):
    x = np.asarray(x, np.float64)
    return np.log1p(np.exp(-np.abs(x))) + np.maximum(x, 0.0)


def _softmax(x):
    e = np.exp(np.asarray(x, np.float64) - np.max(x))
    return e / e.sum()


def _modal_multiplier(f):
    """Combined spectral multiplier M_c: (64, 32, 32) complex128."""
    gh = _softmax(f["seg_h_h"]) * LSEG
    gw = _softmax(f["seg_h_w"]) * LSEG
    seg_r = (np.arange(M1) * LSEG) // M1
    seg_c = (np.arange(M2) * LSEG) // M2
    seg_gain = gh[seg_r][:, None] * gw[seg_c][None, :]

    ky = np.linspace(0.0, 1.0, M1)
    kx = np.linspace(0.0, 1.0, M2)
    Ky, Kx = ky[:, None], kx[None, :]
    r2 = Ky * Ky + Kx * Kx
    r = np.sqrt(r2 + 1e-12)
    nu0 = _softplus(f["nu_log"])
    alpha0 = _softplus(f["alpha_log"])
    c_amp = _softplus(f["c_log"])
    amp_base = np.exp(-nu0 * r2) + c_amp / (1.0 + alpha0 * r2 + 1e-6)
    w0 = (r <= 0.33).astype(np.float64)
    w2b = (r >= 0.66).astype(np.float64)
    w1 = np.maximum(1.0 - w0 - w2b, 0.0)
    g = _softplus(f["band_gain"])
    amp_base = amp_base * ((1.0 + g[0]) * w0 + (1.0 + g[1]) * w1 + (1.0 + g[2]) * w2b)
    phi_base = np.float64(f["omega_y"]) * Ky + np.float64(f["omega_x"]) * Kx

    B_rad = np.stack([r**k for k in range(RADIAL_K)], axis=0)
    amp_delta = np.einsum("ck,khw->chw", _softplus(f["amp_coef"]), B_rad)
    phase_delta = np.einsum(
        "ck,khw->chw", np.asarray(f["phase_coef"], np.float64), B_rad)
    amp_full = amp_base[None] * (1.0 + np.maximum(amp_delta, 0.0))
    phi_full = phi_base[None] + phase_delta
    kernel = (np.cos(phi_full) + 1j * np.sin(phi_full)) * amp_full
    fk = (np.asarray(f["free_kernel_re"], np.float64)
          + 1j * np.asarray(f["free_kernel_im"], np.float64))
    return seg_gain[None] * kernel * SPEC_SCALE * (1.0 + np.float64(f["free_eps"]) * fk)


def _dft_mats():
    hh = np.arange(H)
    fr = np.concatenate([np.arange(HALF_M), np.arange(H - (M1 - HALF_M), H)])
    ang_h = 2.0 * np.pi * np.outer(hh, fr) / H          # (256, 32)
    fhT = np.concatenate([np.cos(ang_h), -np.sin(ang_h)], axis=1)  # (256, 64)

    ww = np.arange(W)
    mm = np.arange(M2)
    ang_w = 2.0 * np.pi * np.outer(ww, mm) / W          # (256, 32)
    FwR, FwI = np.cos(ang_w), -np.sin(ang_w)
    fwA = np.concatenate([FwR, FwI], axis=1)            # (256, 64)
    fwB = np.concatenate([-FwI, FwR], axis=1)           # (256, 64)

    GhR = np.cos(ang_h).T / H                           # (32, 256)
    GhI = np.sin(ang_h).T / H
    ghR2 = np.concatenate([GhR, -GhI], axis=0)          # (64, 256)
    ghI2 = np.concatenate([GhI, GhR], axis=0)           # (64, 256)
    cm = np.full(M2, 2.0)
    cm[0] = 1.0
    GwR = (cm[:, None] * np.cos(ang_w.T)) / W           # (32, 256)
    GwI = (cm[:, None] * np.sin(ang_w.T)) / W
    gw2 = np.concatenate([GwR, -GwI], axis=0)           # (64, 256)
    return fhT, fwA, fwB, ghR2, ghI2, gw2


def _pack_xq8(xo):
    """xo: (32, 256, 256) f32 own channels -> (128, XB_P) fp8 pitch-257."""
    xr = xo.reshape(OC, NQ, 64, 256)                    # (c, q, r, w)
    buf = np.zeros((NQ, OC, XB_P), np.float32)
    body = np.zeros((NQ, OC, 64, WP), np.float32)
    body[..., :256] = xr.transpose(1, 0, 2, 3)
    buf[:, :, HALO_P:HALO_P + FQP] = body.reshape(NQ, OC, FQP)
    # halo row -1 (prev quarter's row 63) at flat [-257, -2]
    buf[1:, :, HALO_P - 257:HALO_P - 1] = xr.transpose(1, 0, 2, 3)[:-1, :, 63]
    # halo row 64 (next quarter's row 0) at flat [FQP, FQP+255]
    buf[:-1, :, HALO_P + FQP:HALO_P + FQP + 256] = \
        xr.transpose(1, 0, 2, 3)[1:, :, 0]
    return buf.reshape(128, XB_P).astype(NP8)


def _per_core_inputs(inputs):
    f = {k: np.asarray(v) for k, v in inputs.items()}
    x = np.asarray(f["x"], np.float32)
    Mc = _modal_multiplier(f)
    fhT, fwA, fwB, ghR2, ghI2, gw2 = _dft_mats()
    gw2e = np.concatenate([gw2, np.ones((1, W))], axis=0)   # (65, 256)

    kd = (SPATIAL_SCALE * np.asarray(f["w_dw3"], np.float64)[:, 0])  # (64, 3, 3)
    w_local = np.asarray(f["w_local"], np.float64)
    w_mlp1 = np.asarray(f["w_mlp1"], np.float64)
    w_mlp2 = np.asarray(f["w_mlp2"], np.float64)
    b_local = np.asarray(f["b_local"], np.float64)
    b_dw3 = np.asarray(f["b_dw3"], np.float64)
    b_mlp1 = np.asarray(f["b_mlp1"], np.float64)
    b_mlp2 = np.asarray(f["b_mlp2"], np.float64)
    gamma = np.asarray(f["gamma"], np.float64)
    beta = np.asarray(f["beta"], np.float64)

    ident16 = np.eye(64, dtype=np.float16)

    qones = np.zeros((128, 32), np.float32)
    for p in range(128):
        qones[p, p % 32] = 1.0
    qonesT = np.ascontiguousarray(qones.T)

    in_maps = []
    for core in range(N_CORES):
        b, half = core // 2, core % 2
        perm = np.concatenate([np.arange(half * 32, half * 32 + 32),
                               np.arange((1 - half) * 32, (1 - half) * 32 + 32)])
        xbv = x[b][perm]                                 # (64, 256, 256)
        oc = perm[:OC]

        xhT = np.ascontiguousarray(
            xbv[:OC].transpose(1, 0, 2).reshape(2, 128, OC * 256)
        ).astype(np.float16)

        xq8 = _pack_xq8(xbv[:OC])

        mcR = np.empty((32, 2, 16, 32), np.float32)
        mcI = np.empty((32, 2, 16, 32), np.float32)
        for ci in range(OC):
            par, pair = ci % 2, ci // 2
            mcR[:, par, pair, :] = Mc[oc[ci]].real.astype(np.float32)
            mcI[:, par, pair, :] = Mc[oc[ci]].imag.astype(np.float32)

        # local 1x1 (+center tap), fp8 q-paired block-diag
        wlocT = (LOCAL_SCALE * w_local[oc][:, perm].T)   # (64c_in, 32oc)
        for ci in range(OC):
            wlocT[ci, ci] += kd[oc[ci], 1, 1]
        wlocT2 = np.zeros((128, 64), np.float64)
        wlocT2[0:64, 0:32] = wlocT
        wlocT2[64:128, 32:64] = wlocT

        wm1T = w_mlp1[:, perm].T
        wm1T2 = np.concatenate([wm1T, wm1T], axis=0)  # (128, 128) dup
        wm2T = w_mlp2[oc].T

        # fp8 DoubleRow tap weights: (128, pair, lane, 128) diag
        ktp8 = np.zeros((128, len(TAP_PAIRS), 2, 128), NP8)
        for pr, (d0, d1) in enumerate(TAP_PAIRS):
            for ln, d in enumerate((d0, d1)):
                dy, dx = _tap_dyx(d)
                kp = np.tile(kd[oc, dy + 1, dx + 1], NQ)
                for p in range(128):
                    ktp8[p, pr, ln, p] = kp[p]

        bconst = (LOCAL_SCALE * b_local[oc] + SPATIAL_SCALE * b_dw3[oc] + b_mlp2[oc])
        bc_row = np.ascontiguousarray(
            np.broadcast_to(bconst[None, :, None], (NQ, OC, 64)).reshape(1, 8192)
        ).astype(np.float16)

        in_maps.append({
            "xhT": xhT,
            "xq8": xq8.view(np.uint8),
            "xbh": np.ascontiguousarray(xbv.astype(np.float16)),

            "fhT": fhT.astype(np.float16),
            "fwA": fwA.astype(np.float16), "fwB": fwB.astype(np.float16),
            "ghR2": ghR2.astype(np.float16), "ghI2": ghI2.astype(np.float16),
            "gw2e": gw2e.astype(np.float16),
            "mcR": mcR, "mcI": mcI,
            "ident16": ident16,
            "wlocT2": wlocT2.astype(np.float16),
            "wm1T2": wm1T2.astype(np.float16),
            "wm2T": wm2T.astype(np.float16),
            "ktp8": ktp8.view(np.uint8),
            "bc_row": bc_row,
            "bm1": b_mlp1.astype(np.float32)[:, None],
            "gam": gamma[oc].astype(np.float32)[:, None],
            "bet": beta[oc].astype(np.float32)[:, None],
            "qones": qones, "qonesT": qonesT,
        })
    return in_maps


def _assemble(x, results):
    """Add residual on host: out = x + gelu-result (fp16, quarter-major)."""
    out = np.empty_like(x)
    for core in range(N_CORES):
        b, half = core // 2, core % 2
        r = np.asarray(results[core]["outp"])
        if r.dtype != np.float16:
            r = r.view(np.float16)
        g = r.reshape(NQ, OC, 64, 256).transpose(1, 0, 2, 3).reshape(
            OC, 256, 256).astype(np.float32)
        sl = slice(half * 32, half * 32 + 32)
        out[b, sl] = x[b, sl] + g
    return out


# ---------------------------------------------------------------- device code
def _build_program():
    nc = bacc.Bacc(None, target_bir_lowering=False, debug=False)
    P = {}

    def di(name, shape, dtype=F32):
        P[name] = nc.declare_dram_parameter(name, list(shape), dtype, isOutput=False)

    di("xhT", (2, 128, OC * 256), FP16)
    di("xq8", (128, XB_P), FP8)
    di("xbh", (C, H, W), FP16)
    di("fhT", (256, 64), FP16)
    di("fwA", (256, 64), FP16); di("fwB", (256, 64), FP16)
    di("ghR2", (64, 256), FP16); di("ghI2", (64, 256), FP16)
    di("gw2e", (65, 256), FP16)
    di("mcR", (32, 2, 16, 32)); di("mcI", (32, 2, 16, 32))
    di("ident16", (64, 64), FP16)
    di("wlocT2", (128, 64), FP16)
    di("wm1T2", (128, 128), FP16)
    di("wm2T", (128, 32), FP16)
    di("ktp8", (128, len(TAP_PAIRS), 2, 128), FP8)
    di("bc_row", (1, 8192), FP16)
    di("bm1", (128, 1))
    di("gam", (32, 1)); di("bet", (32, 1))
    di("qones", (128, 32)); di("qonesT", (32, 128))
    outp = nc.declare_dram_parameter("outp", [128, FQ], FP16, isOutput=True)

    with tile.TileContext(nc) as tc, ExitStack() as ctx:
        _body(ctx, tc, P, outp)
    nc.finalize()
    return nc


def _body(ctx, tc, P, outp):
    nc = tc.nc
    xbh_f = P["xbh"].rearrange("c h w -> c (h w)")       # (64, 65536) fp16

    consts = ctx.enter_context(tc.tile_pool(name="consts", bufs=1))

    def load_const(name, shape, dtype=F32):
        t = consts.tile(list(shape), dtype, tag=name)
        nc.sync.dma_start(out=t, in_=P[name][:])
        return t

    main = ctx.enter_context(tc.tile_pool(name="main", bufs=1))
    mid = ctx.enter_context(tc.tile_pool(name="mid", bufs=1))

    # ---------------- S1 + S2: spectral (pool also scopes init DMAs) ----
    fhT_s = consts.tile([128, 2, 64], FP16, tag="fhT")
    nc.sync.dma_start(out=fhT_s, in_=P["fhT"].rearrange("(t p) m -> p t m", p=128))

    with tc.tile_pool(name="spec1", bufs=1) as sp1, \
         tc.tile_pool(name="xhp", bufs=4) as xhp:
        xhbs = []
        for blk in range(4):
            xhb = xhp.tile([128, 2, 2048], FP16, tag="xhb")
            for ht in range(2):
                nc.sync.dma_start(
                    out=xhb[:, ht, :],
                    in_=P["xhT"][ht, :, blk * 2048:(blk + 1) * 2048])
            xhbs.append(xhb)

        fwA_s = consts.tile([128, 2, 64], FP16, tag="fwA")
        nc.sync.dma_start(out=fwA_s, in_=P["fwA"].rearrange("(t p) m -> p t m", p=128))
        fwB_s = consts.tile([128, 2, 64], FP16, tag="fwB")
        nc.sync.dma_start(out=fwB_s, in_=P["fwB"].rearrange("(t p) m -> p t m", p=128))
        ident_s = load_const("ident16", (64, 64), FP16)
        ghR2_s = load_const("ghR2", (64, 256), FP16)
        ghI2_s = load_const("ghI2", (64, 256), FP16)
        gw2e_s = load_const("gw2e", (65, 256), FP16)

        xq8 = main.tile([128, XB_P], FP8, tag="xq8")
        for s in range(2):
            c0 = s * (XB_P // 2)
            c1 = XB_P if s else (XB_P // 2)
            nc.sync.dma_start(out=xq8[:, c0:c1], in_=P["xq8"][:, c0:c1])
        ktp8_s = consts.tile([128, len(TAP_PAIRS), 2, 128], FP8, tag="ktp8")
        nc.sync.dma_start(out=ktp8_s, in_=P["ktp8"][:])
        wlocT2_s = load_const("wlocT2", (128, 64), FP16)
        wm1T2_s = load_const("wm1T2", (128, 128), FP16)
        wm2T_s = load_const("wm2T", (128, 32), FP16)
        mcR_s = load_const("mcR", (32, 2, 16, 32))
        mcI_s = load_const("mcI", (32, 2, 16, 32))
        bm1_s = load_const("bm1", (128, 1))
        gam_s = load_const("gam", (32, 1))
        bet_s = load_const("bet", (32, 1))
        qones_s = load_const("qones", (128, 32))
        qonesT_s = load_const("qonesT", (32, 128))

        QstA = mid.tile([64, 8, 2, 32], FP16, tag="QstA")
        QstB = mid.tile([64, 8, 2, 32], FP16, tag="QstB")
        Qsth = [QstA, QstB]
        Qst_ch = [QstA.rearrange("p a b w -> p (a b) w"),
                  QstB.rearrange("p a b w -> p (a b) w")]        # (64, 16, 32)
        Zh2e = mid.tile([65, 4, 32, 64], FP16, tag="Zh2e")
        nc.sync.dma_start(out=Zh2e[64:65, :, :, :],
                          in_=P["bc_row"].rearrange("p (q c l) -> p q c l",
                                                    q=4, c=32))

        zbuf = main.tile([128, FQ], FP16, tag="zbuf")
        szc = main.tile([128, NHC], F32, tag="szc")
        sqc = main.tile([128, NHC], F32, tag="sqc")

        T1 = sp1.tile([64, OC, 256], FP16, tag="T1")             # (rmRI, c, w)
        T1v = T1.rearrange("p c w -> p (c w)")
        # PE warm-up while DMAs land (output read once, then overwritten)
        with tc.tile_pool(name="ps_warm", bufs=1, space="PSUM") as ps_w:
            wps = ps_w.tile([64, 128], F32, tag="warm")
            fhflat = fhT_s.rearrange("p t m -> p (t m)")
            for wi in range(40):
                nc.tensor.matmul(out=wps, lhsT=fhT_s[:, 0, :], rhs=fhflat,
                                 start=True, stop=True)
            nc.scalar.copy(out=T1v[:, 0:128], in_=wps)
        T1T0 = sp1.tile([128, 2, OC, 32], FP16, tag="T1T0")
        T1T1 = sp1.tile([128, 2, OC, 32], FP16, tag="T1T1")
        T1T = [T1T0, T1T1]

        with tc.tile_pool(name="ps_t1", bufs=2, space="PSUM") as ps_t1, \
             tc.tile_pool(name="ps_tr", bufs=2, space="PSUM") as ps_tr:
            for blk in range(4):
                xhb = xhbs[blk]
                for sub in range(2):
                    reg = blk * 2 + sub
                    pt = ps_t1.tile([64, 1024], F32, tag="t1p")
                    for _w in range(6):
                        nc.tensor.matmul(out=pt[:, 0:64],
                                         lhsT=fhT_s[:, 0, :],
                                         rhs=fhT_s[:, 0, :],
                                         start=True, stop=True)
                    for n in range(2):
                        col = sub * 1024 + n * 512
                        for ht in range(2):
                            nc.tensor.matmul(
                                out=pt[:, n * 512:(n + 1) * 512],
                                lhsT=fhT_s[:, ht, :],
                                rhs=xhb[:, ht, col:col + 512],
                                start=(ht == 0), stop=(ht == 1))
                    nc.scalar.copy(out=T1v[:, reg * 1024:(reg + 1) * 1024],
                                   in_=pt)

            for wh in range(2):
                for cb in range(4):
                    pt2 = ps_tr.tile([128, 512], FP16, tag="trp")
                    for i in range(8):
                        cch = cb * 8 + i
                        nc.tensor.transpose(
                            out=pt2[:, i * 64:(i + 1) * 64],
                            in_=T1[:, cch, wh * 128:(wh + 1) * 128],
                            identity=ident_s)
                    # one fused strided copy per (wh, cb) on the DVE --
                    # keeps the Act free so part1(0/1) gelus start early
                    ptv2 = pt2.rearrange("p (c a b) -> p a c b", c=8, a=2)
                    nc.vector.tensor_copy(
                        out=T1T[wh][:, :, cb * 8:(cb + 1) * 8, :],
                        in_=ptv2)

        # ---- S2 (cp/Qst/Zh) shares psum pools with the loop; part1(0/1)
        # interleave into the Zh stage to keep the PE warm through the
        # spectral tail and hide Qst/copy latencies.
        with tc.tile_pool(name="xqp", bufs=4) as xqp, \
             tc.tile_pool(name="h1sp", bufs=8) as h1sp, \
             tc.tile_pool(name="sqscr", bufs=2) as sqscr, \
             tc.tile_pool(name="ps_zp", bufs=2, space="PSUM") as ps_zp, \
             tc.tile_pool(name="ps_h1", bufs=2, space="PSUM") as ps_h1:
            ZPs = [None] * NHC
            h1ss = [None] * NHC
            xqt = [None] * (NHC // 2)

            def tap_rhs(pr, base):
                d0, d1 = TAP_PAIRS[pr]
                return APc(xq8.tensor, xq8.offset + base + d0,
                           [list(xq8.ap[0]), [d1 - d0, 2], [WP, 2], [1, 256]])

            def mlp1(hc, q):
                """mlp1 quarter q into a fresh psum buffer + immediate gelu."""
                xq2 = xqt[hc // 2]
                xoff = (hc % 2) * HC
                j = q % 2
                hp = ps_h1.tile([128, HC], F32, tag="h1p")
                for s2 in range(0, HC, 512):
                    nc.tensor.matmul(
                        out=hp[:, s2:s2 + 512],
                        lhsT=wm1T2_s[64 * j:64 * j + 64, :],
                        rhs=xq2[q // 2][64 * j:64 * j + 64,
                                        xoff + s2:xoff + s2 + 512],
                        start=True, stop=True, tile_position=(64 * j, 0))
                h1s = h1sp.tile([128, HC], FP16, tag="h1s")
                nc.scalar.activation(out=h1s, in_=hp, func=AF.Gelu,
                                     bias=bm1_s, scale=1.0)
                h1ss[hc].append(h1s)

            def tap(hc, t8):
                rg, pr = t8 // 4, t8 % 4
                ZPv = ZPs[hc].rearrange("p (g a b) -> p g a b", g=2, a=2)
                base = HALO_P + WP * (4 * hc + 2 * rg)
                nc.tensor.matmul(
                    out=ZPv[:, rg, :, :], lhsT=ktp8_s[:, pr, :, :],
                    rhs=tap_rhs(pr, base),
                    start=False, stop=False, skip_group_check=True,
                    perf_mode=mybir.MatmulPerfMode.DoubleRow)

            def part1(hc):
                f0 = hc * HC
                ZP = ps_zp.tile([128, HC], F32, tag="ZP")
                ZPs[hc] = ZP
                h1ss[hc] = []
                if hc % 2 == 0:
                    xq = []
                    for qp in range(2):
                        t = xqp.tile([128, 2 * HC], FP16, tag="xq")
                        for j in range(2):
                            q = qp * 2 + j
                            nc.sync.dma_start(
                                out=t[64 * j:64 * j + 64, :],
                                in_=xbh_f[:, q * FQ + f0:q * FQ + f0 + 2 * HC])
                        xq.append(t)
                    xqt[hc // 2] = xq
                xq2 = xqt[hc // 2]
                xoff = (hc % 2) * HC

                mlp1(hc, 0)
                mlp1(hc, 1)
                # local (q-paired block-diag): start=True
                for qp in range(2):
                    tp = (0, 64 * qp) if qp > 0 else None
                    for s in range(0, HC, 512):
                        nc.tensor.matmul(
                            out=ZP[64 * qp:64 * qp + 64, s:s + 512],
                            lhsT=wlocT2_s,
                            rhs=xq2[qp][:, xoff + s:xoff + s + 512],
                            start=True, stop=False, skip_group_check=True,
                            tile_position=tp)
                for t5 in range(5):
                    tap(hc, t5)

            def part2a(hc):
                for t8 in range(5, 8):
                    tap(hc, t8)
                mlp1(hc, 2)
                mlp1(hc, 3)

            def part2b(hc):
                ZP = ZPs[hc]
                for i in range(4):
                    h0 = hc * 4 + i
                    nc.tensor.matmul(out=ZP[:, i * 256:(i + 1) * 256],
                                     lhsT=Zh2e[:, :, :, h0], rhs=gw2e_s,
                                     start=False, stop=False,
                                     skip_group_check=True)

            def part2(hc):
                part2b(hc)
                part2a(hc)

            def finish(hc):
                f0 = hc * HC
                ZP = ZPs[hc]
                for q in range(NQ):
                    tp = (0, 32 * q) if q > 0 else None
                    for s in range(0, HC, 512):
                        nc.tensor.matmul(out=ZP[32 * q:32 * q + 32, s:s + 512],
                                         lhsT=wm2T_s,
                                         rhs=h1ss[hc][q][:, s:s + 512],
                                         start=False, stop=True,
                                         tile_position=tp,
                                         skip_group_check=True)
                # zbuf = ZP (fp16), accumulate sum into szc (DVE)
                nc.vector.tensor_scalar(
                    out=zbuf[:, f0:f0 + HC], in0=ZP, scalar1=0.0, scalar2=0.0,
                    op0=AX.add, op1=AX.add, accum_out=szc[:, hc:hc + 1])
                # square + sum; last hc on DVE to shorten the tail chain
                scr = sqscr.tile([128, HC], FP16, tag="scr")
                if hc == NHC - 1:
                    nc.vector.tensor_tensor(out=scr, in0=zbuf[:, f0:f0 + HC],
                                            in1=zbuf[:, f0:f0 + HC],
                                            op=AX.mult)
                else:
                    nc.gpsimd.tensor_tensor(out=scr, in0=zbuf[:, f0:f0 + HC],
                                            in1=zbuf[:, f0:f0 + HC],
                                            op=AX.mult)
                nc.vector.tensor_reduce(out=sqc[:, hc:hc + 1], in_=scr,
                                        axis=mybir.AxisListType.X, op=AX.add)

            # ---- cp stage: psum views carved from full-size ps_h1 tiles ----
            cpt = [ps_h1.tile([128, 1024], F32, tag="h1p",
                              name=f"cpt{_i}") for _i in range(2)]
            cph = [t[0:64, 0:512].rearrange("p (a b c) -> p a b c", a=8, b=2)
                   for t in cpt]
            tmpA = sp1.tile([32, 8, 32], F32, tag="mtmpA")
            tmpB = sp1.tile([32, 8, 32], F32, tag="mtmpB")
            for half in range(2):
                cp = cph[half]
                for pr in range(8 * half, 8 * half + 8):
                    dst = cp[:, pr - 8 * half, :, :].rearrange(
                        "p a b -> p (a b)")
                    for wh in range(2):
                        nc.tensor.matmul(out=dst,
                                         lhsT=T1T[wh][:, 0, 2 * pr:2 * pr + 2, :],
                                         rhs=fwA_s[:, wh, :],
                                         start=(wh == 0), stop=False)
                    for wh in range(2):
                        nc.tensor.matmul(out=dst,
                                         lhsT=T1T[wh][:, 1, 2 * pr:2 * pr + 2, :],
                                         rhs=fwB_s[:, wh, :],
                                         start=False, stop=(wh == 1))
                prs = slice(8 * half, 8 * half + 8)
                Qh = Qsth[half]
                for par in range(2):
                    crs = cp[32 * par:32 * par + 32, :, 0, :]
                    cis = cp[32 * par:32 * par + 32, :, 1, :]
                    mr = mcR_s[:, par, prs, :]
                    mi = mcI_s[:, par, prs, :]
                    nc.vector.tensor_tensor(out=tmpA, in0=crs, in1=mr, op=AX.mult)
                    nc.vector.tensor_tensor(out=tmpB, in0=cis, in1=mi, op=AX.mult)
                    nc.vector.tensor_tensor(out=Qh[0:32, :, par, :],
                                            in0=tmpA, in1=tmpB, op=AX.subtract)
                    nc.vector.tensor_tensor(out=tmpA, in0=cis, in1=mr, op=AX.mult)
                    nc.vector.tensor_tensor(out=tmpB, in0=crs, in1=mi, op=AX.mult)
                    nc.vector.tensor_tensor(out=Qh[32:64, :, par, :],
                                            in0=tmpA, in1=tmpB, op=AX.add)

            def zh(grp):
                t = ps_zp.tile([128, 1024], F32, tag="ZP", name=f"zh{grp}")
                zp = t[0:64, :].rearrange("p (c l) -> p c l", c=4)
                for i in range(4):
                    cch = grp * 4 + i
                    lhs = Qst_ch[grp // 4][:, cch - 16 * (grp // 4), :]
                    nc.tensor.matmul(out=zp[0:32, i, :], lhsT=lhs, rhs=ghR2_s,
                                     start=True, stop=True)
                    nc.tensor.matmul(out=zp[32:64, i, :], lhsT=lhs, rhs=ghI2_s,
                                     start=True, stop=True,
                                     tile_position=(0, 32))
                nc.scalar.copy(
                    out=Zh2e[0:64, :, grp * 4:(grp + 1) * 4, :],
                    in_=zp.rearrange("p c (q l) -> p q c l", q=4))

            for g in range(8):
                zh(g)
            part1(0)
            part1(1)

            for hc in range(NHC + 1):
                if 2 <= hc < NHC:
                    part1(hc)
                if hc >= 1:
                    finish(hc - 1)
                if hc < NHC:
                    part2(hc)

    # ---------------- S5: stats ----------------
    st = ctx.enter_context(tc.tile_pool(name="stats", bufs=1))
    with tc.tile_pool(name="ps_st", bufs=1, space="PSUM") as ps_st:
        sums = st.tile([128, 2], F32, tag="sums")
        nc.vector.tensor_reduce(out=sums[:, 0:1], in_=szc,
                                axis=mybir.AxisListType.X, op=AX.add)
        nc.vector.tensor_reduce(out=sums[:, 1:2], in_=sqc,
                                axis=mybir.AxisListType.X, op=AX.add)
        sp = ps_st.tile([32, 2], F32, tag="sp")
        nc.tensor.matmul(out=sp, lhsT=qones_s, rhs=sums, start=True, stop=True)
        mu = st.tile([32, 1], F32, tag="mu")
        negmu = st.tile([32, 1], F32, tag="negmu")
        ex2 = st.tile([32, 1], F32, tag="ex2")
        var = st.tile([32, 1], F32, tag="var")
        s12 = st.tile([32, 2], F32, tag="s12")
        inv_n = 1.0 / float(HW)
        nc.vector.tensor_scalar(out=mu, in0=sp[:, 0:1], scalar1=inv_n,
                                scalar2=None, op0=AX.mult)
        nc.vector.tensor_scalar(out=negmu, in0=sp[:, 0:1], scalar1=-inv_n,
                                scalar2=None, op0=AX.mult)
        nc.vector.tensor_scalar(out=ex2, in0=sp[:, 1:2], scalar1=inv_n,
                                scalar2=None, op0=AX.mult)
        nc.vector.scalar_tensor_tensor(out=var, in0=mu, scalar=negmu, in1=ex2,
                                       op0=AX.mult, op1=AX.add)
        epst = st.tile([32, 1], F32, tag="epst")
        nc.vector.memset(epst, 1e-5)
        nc.scalar.activation(out=var, in_=var, func=AF.Sqrt, bias=epst, scale=1.0)
        nc.vector.reciprocal(out=var, in_=var)                   # rstd
        nc.vector.tensor_tensor(out=s12[:, 0:1], in0=var, in1=gam_s, op=AX.mult)
        nc.vector.tensor_scalar(out=negmu, in0=mu, scalar1=-1.0,
                                scalar2=None, op0=AX.mult)
        nc.vector.scalar_tensor_tensor(out=s12[:, 1:2], in0=s12[:, 0:1],
                                       scalar=negmu, in1=bet_s,
                                       op0=AX.mult, op1=AX.add)
        spb = ps_st.tile([128, 2], F32, tag="spb")
        nc.tensor.matmul(out=spb, lhsT=qonesT_s, rhs=s12, start=True, stop=True)
        s12s = st.tile([128, 2], F32, tag="s12s")
        nc.vector.tensor_copy(out=s12s, in_=spb)

    # ---------------- S6: sweep 2 (gelu only; +x residual on host) ----
    with tc.tile_pool(name="sw2g", bufs=3) as sw2g:
        for ch in range(4):
            f0 = ch * 4096
            g = sw2g.tile([128, 4096], FP16, tag="g")
            nc.scalar.activation(out=g, in_=zbuf[:, f0:f0 + 4096], func=AF.Gelu,
                                 bias=s12s[:, 1:2], scale=s12s[:, 0:1])
            for s in range(0, 4096, 2048):
                nc.sync.dma_start(out=outp[:, f0 + s:f0 + s + 2048],
                                  in_=g[:, s:s + 2048])


_PROGRAM = None


def kernel(**inputs):
    global _PROGRAM
    in_maps = _per_core_inputs(inputs)
    if _PROGRAM is None:
        _PROGRAM = _build_program()
    res = run_bass_kernel_spmd(_PROGRAM, in_maps, list(range(N_CORES)))
    x = np.asarray(inputs["x"], np.float32)
    return _assemble(x, res.results)


# revision 37
# speedup vs baseline: 1.2374x; 1.2374x over previous
"""Trainium2 Bass kernel for nn_CFTAOBlock2D (v17, ~169us vs 214us v3).

Sharding: pure data-parallel over (batch b, channel-half) -> 8 cores.

Key structure:
  - all 8 depthwise 3x3 taps run on PE as 4 fp8e4m3 DoubleRow
    pair-matmuls against a pitch-257 (1 zero pad col per row) fp8 copy
    of x (xq8). The pad column zeroes edge-column reads -> no boundary
    corrections and no misaligned DVE tap ops. DoubleRow rhs lanes are
    two offset slices of one buffer via a hand-built 4D AP.
  - local 1x1 + mlp1 stay fp16 (fp8 there costs ~1.8e-2 rel err; taps
    in fp8 cost only ~3e-4), streamed per-hc from channel-major xbh.
  - per-quarter mlp1 psum tiles with immediate per-1024 gelus on Act;
    zbuf merge (+szc accum) on DVE, squares on gpsimd (DVE for the
    last chunk), sum-reduce on DVE. Act stays gelu-only in the loop.
  - spectral cp shares the loop's h1 psum pool and Zh stages run from
    the (still free) ZP pool BEFORE part1(0/1) -- keeping the zh tile
    rotation off the Act-gelu dependency chain; transpose staging is
    one fused strided DVE copy per block so the Act drains its T1
    copies early.
  - output is gelu-only fp16 (4096-wide final sweep); the +x residual
    is added on host in f32.
  - known limits: steady loop is Act-chain bound (~5.6us/chunk); the
    head cold-clock window (~27-51us) resists reordering; runs vary
    ~3% with device P0 state (up to +20% right after a hot run).
"""
from contextlib import ExitStack

import numpy as np
import ml_dtypes

import bass_rust
import concourse.bass as bass
import concourse.bacc as bacc
import concourse.tile as tile
from concourse import mybir
from concourse.bass_utils import run_bass_kernel_spmd

F32 = mybir.dt.float32
FP16 = mybir.dt.float16
FP8 = mybir.dt.float8e4
NP8 = ml_dtypes.float8_e4m3
AX = mybir.AluOpType
AF = mybir.ActivationFunctionType
APc = bass_rust.AP

B, C, H, W = 4, 64, 256, 256
M1, M2 = 32, 32
HALF_M = 16
LSEG, RADIAL_K = 4, 4
LOCAL_SCALE, SPATIAL_SCALE, SPEC_SCALE = 0.3, 0.15, 1.0
HW = H * W            # 65536
NQ = 4                # H quarters
FQ = 16384            # dense free per quarter
NHC = 16              # half-chunks of 1024 cols
HC = 1024
OC = 32               # own channels per core
N_CORES = 8

# pitch-257 packed geometry for the tap buffer
WP = 257              # row pitch (256 data + 1 zero pad)
FQP = 64 * WP         # 16448 per quarter
HALO_P = 258
XB_P = FQP + 2 * HALO_P   # 16964

# DoubleRow tap pairs: (d0, d1) flat offsets at pitch 257
TAP_PAIRS = [(-258, 258), (-257, -1), (-256, 256), (1, 257)]


def _tap_dyx(d):
    # d = 257*dy + dx with dy in {-1,0,1}, dx in {-1,0,1}
    for dy in (-1, 0, 1):
        dx = d - 257 * dy
        if -1 <= dx <= 1:
            return dy, dx
    raise ValueError(d)


# ---------------------------------------------------------------- host math
def _softplus(x):
    x = np.asarray(x, np.float64)
    return np.log1p(np.exp(-np.abs(x))) + np.maximum(x, 0.0)


def _softmax(x):
    e = np.exp(np.asarray(x, np.float64) - np.max(x))
    return e / e.sum()


def _modal_multiplier(f):
    """Combined spectral multiplier M_c: (64, 32, 32) complex128."""
    gh = _softmax(f["seg_h_h"]) * LSEG
    gw = _softmax(f["seg_h_w"]) * LSEG
    seg_r = (np.arange(M1) * LSEG) // M1
    seg_c = (np.arange(M2) * LSEG) // M2
    seg_gain = gh[seg_r][:, None] * gw[seg_c][None, :]

    ky = np.linspace(0.0, 1.0, M1)
    kx = np.linspace(0.0, 1.0, M2)
    Ky, Kx = ky[:, None], kx[None, :]
    r2 = Ky * Ky + Kx * Kx
    r = np.sqrt(r2 + 1e-12)
    nu0 = _softplus(f["nu_log"])
    alpha0 = _softplus(f["alpha_log"])
    c_amp = _softplus(f["c_log"])
    amp_base = np.exp(-nu0 * r2) + c_amp / (1.0 + alpha0 * r2 + 1e-6)
    w0 = (r <= 0.33).astype(np.float64)
    w2b = (r >= 0.66).astype(np.float64)
    w1 = np.maximum(1.0 - w0 - w2b, 0.0)
    g = _softplus(f["band_gain"])
    amp_base = amp_base * ((1.0 + g[0]) * w0 + (1.0 + g[1]) * w1 + (1.0 + g[2]) * w2b)
    phi_base = np.float64(f["omega_y"]) * Ky + np.float64(f["omega_x"]) * Kx

    B_rad = np.stack([r**k for k in range(RADIAL_K)], axis=0)
    amp_delta = np.einsum("ck,khw->chw", _softplus(f["amp_coef"]), B_rad)
    phase_delta = np.einsum(
        "ck,khw->chw", np.asarray(f["phase_coef"], np.float64), B_rad)
    amp_full = amp_base[None] * (1.0 + np.maximum(amp_delta, 0.0))
    phi_full = phi_base[None] + phase_delta
    kernel = (np.cos(phi_full) + 1j * np.sin(phi_full)) * amp_full
    fk = (np.asarray(f["free_kernel_re"], np.float64)
          + 1j * np.asarray(f["free_kernel_im"], np.float64))
    return seg_gain[None] * kernel * SPEC_SCALE * (1.0 + np.float64(f["free_eps"]) * fk)


def _dft_mats():
    hh = np.arange(H)
    fr = np.concatenate([np.arange(HALF_M), np.arange(H - (M1 - HALF_M), H)])
    ang_h = 2.0 * np.pi * np.outer(hh, fr) / H          # (256, 32)
    fhT = np.concatenate([np.cos(ang_h), -np.sin(ang_h)], axis=1)  # (256, 64)

    ww = np.arange(W)
    mm = np.arange(M2)
    ang_w = 2.0 * np.pi * np.outer(ww, mm) / W          # (256, 32)
    FwR, FwI = np.cos(ang_w), -np.sin(ang_w)
    fwA = np.concatenate([FwR, FwI], axis=1)            # (256, 64)
    fwB = np.concatenate([-FwI, FwR], axis=1)           # (256, 64)

    GhR = np.cos(ang_h).T / H                           # (32, 256)
    GhI = np.sin(ang_h).T / H
    ghR2 = np.concatenate([GhR, -GhI], axis=0)          # (64, 256)
    ghI2 = np.concatenate([GhI, GhR], axis=0)           # (64, 256)
    cm = np.full(M2, 2.0)
    cm[0] = 1.0
    GwR = (cm[:, None] * np.cos(ang_w.T)) / W           # (32, 256)
    GwI = (cm[:, None] * np.sin(ang_w.T)) / W
    gw2 = np.concatenate([GwR, -GwI], axis=0)           # (64, 256)
    return fhT, fwA, fwB, ghR2, ghI2, gw2


def _pack_xq8(xo):
    """xo: (32, 256, 256) f32 own channels -> (128, XB_P) fp8 pitch-257."""
    xr = xo.reshape(OC, NQ, 64, 256)                    # (c, q, r, w)
    buf = np.zeros((NQ, OC, XB_P), np.float32)
    body = np.zeros((NQ, OC, 64, WP), np.float32)
    body[..., :256] = xr.transpose(1, 0, 2, 3)
    buf[:, :, HALO_P:HALO_P + FQP] = body.reshape(NQ, OC, FQP)
    # halo row -1 (prev quarter's row 63) at flat [-257, -2]
    buf[1:, :, HALO_P - 257:HALO_P - 1] = xr.transpose(1, 0, 2, 3)[:-1, :, 63]
    # halo row 64 (next quarter's row 0) at flat [FQP, FQP+255]
    buf[:-1, :, HALO_P + FQP:HALO_P + FQP + 256] = \
        xr.transpose(1, 0, 2, 3)[1:, :, 0]
    return buf.reshape(128, XB_P).astype(NP8)


def _per_core_inputs(inputs):
    f = {k: np.asarray(v) for k, v in inputs.items()}
    x = np.asarray(f["x"], np.float32)
    Mc = _modal_multiplier(f)
    fhT, fwA, fwB, ghR2, ghI2, gw2 = _dft_mats()
    gw2e = np.concatenate([gw2, np.ones((1, W))], axis=0)   # (65, 256)

    kd = (SPATIAL_SCALE * np.asarray(f["w_dw3"], np.float64)[:, 0])  # (64, 3, 3)
    w_local = np.asarray(f["w_local"], np.float64)
    w_mlp1 = np.asarray(f["w_mlp1"], np.float64)
    w_mlp2 = np.asarray(f["w_mlp2"], np.float64)
    b_local = np.asarray(f["b_local"], np.float64)
    b_dw3 = np.asarray(f["b_dw3"], np.float64)
    b_mlp1 = np.asarray(f["b_mlp1"], np.float64)
    b_mlp2 = np.asarray(f["b_mlp2"], np.float64)
    gamma = np.asarray(f["gamma"], np.float64)
    beta = np.asarray(f["beta"], np.float64)

    ident16 = np.eye(64, dtype=np.float16)

    qones = np.zeros((128, 32), np.float32)
    for p in range(128):
        qones[p, p % 32] = 1.0
    qonesT = np.ascontiguousarray(qones.T)

    in_maps = []
    for core in range(N_CORES):
        b, half = core // 2, core % 2
        perm = np.concatenate([np.arange(half * 32, half * 32 + 32),
                               np.arange((1 - half) * 32, (1 - half) * 32 + 32)])
        xbv = x[b][perm]                                 # (64, 256, 256)
        oc = perm[:OC]

        xhT = np.ascontiguousarray(
            xbv[:OC].transpose(1, 0, 2).reshape(2, 128, OC * 256)
        ).astype(np.float16)

        xq8 = _pack_xq8(xbv[:OC])

        mcR = np.empty((32, 2, 16, 32), np.float32)
        mcI = np.empty((32, 2, 16, 32), np.float32)
        for ci in range(OC):
            par, pair = ci % 2, ci // 2
            mcR[:, par, pair, :] = Mc[oc[ci]].real.astype(np.float32)
            mcI[:, par, pair, :] = Mc[oc[ci]].imag.astype(np.float32)

        # local 1x1 (+center tap), fp8 q-paired block-diag
        wlocT = (LOCAL_SCALE * w_local[oc][:, perm].T)   # (64c_in, 32oc)
        for ci in range(OC):
            wlocT[ci, ci] += kd[oc[ci], 1, 1]
        wlocT2 = np.zeros((128, 64), np.float64)
        wlocT2[0:64, 0:32] = wlocT
        wlocT2[64:128, 32:64] = wlocT

        wm1T = w_mlp1[:, perm].T
        wm1T2 = np.concatenate([wm1T, wm1T], axis=0)  # (128, 128) dup
        wm2T = w_mlp2[oc].T

        # fp8 DoubleRow tap weights: (128, pair, lane, 128) diag
        ktp8 = np.zeros((128, len(TAP_PAIRS), 2, 128), NP8)
        for pr, (d0, d1) in enumerate(TAP_PAIRS):
            for ln, d in enumerate((d0, d1)):
                dy, dx = _tap_dyx(d)
                kp = np.tile(kd[oc, dy + 1, dx + 1], NQ)
                for p in range(128):
                    ktp8[p, pr, ln, p] = kp[p]

        bconst = (LOCAL_SCALE * b_local[oc] + SPATIAL_SCALE * b_dw3[oc] + b_mlp2[oc])
        bc_row = np.ascontiguousarray(
            np.broadcast_to(bconst[None, :, None], (NQ, OC, 64)).reshape(1, 8192)
        ).astype(np.float16)

        in_maps.append({
            "xhT": xhT,
            "xq8": xq8.view(np.uint8),
            "xbh": np.ascontiguousarray(xbv.astype(np.float16)),

            "fhT": fhT.astype(np.float16),
            "fwA": fwA.astype(np.float16), "fwB": fwB.astype(np.float16),
            "ghR2": ghR2.astype(np.float16), "ghI2": ghI2.astype(np.float16),
            "gw2e": gw2e.astype(np.float16),
            "mcR": mcR, "mcI": mcI,
            "ident16": ident16,
            "wlocT2": wlocT2.astype(np.float16),
            "wm1T2": wm1T2.astype(np.float16),
            "wm2T": wm2T.astype(np.float16),
            "ktp8": ktp8.view(np.uint8),
            "bc_row": bc_row,
            "bm1": b_mlp1.astype(np.float32)[:, None],
            "gam": gamma[oc].astype(np.float32)[:, None],
            "bet": beta[oc].astype(np.float32)[:, None],
            "qones": qones, "qonesT": qonesT,
        })
    return in_maps


def _assemble(x, results):
    """Add residual on host: out = x + gelu-result (fp16, quarter-major)."""
    out = np.empty_like(x)
    for core in range(N_CORES):
        b, half = core // 2, core % 2
        r = np.asarray(results[core]["outp"])
        if r.dtype != np.float16:
            r = r.view(np.float16)
        g = r.reshape(NQ, OC, 64, 256).transpose(1, 0, 2, 3).reshape(
            OC, 256, 256).astype(np.float32)
        sl = slice(half * 32, half * 32 + 32)
        out[b, sl] = x[b, sl] + g
    return out


# ---------------------------------------------------------------- device code
def _build_program():
    nc = bacc.Bacc(None, target_bir_lowering=False, debug=False)
    P = {}

    def di(name, shape, dtype=F32):
        P[name] = nc.declare_dram_parameter(name, list(shape), dtype, isOutput=False)

    di("xhT", (2, 128, OC * 256), FP16)
    di("xq8", (128, XB_P), FP8)
    di("xbh", (C, H, W), FP16)
    di("fhT", (256, 64), FP16)
    di("fwA", (256, 64), FP16); di("fwB", (256, 64), FP16)
    di("ghR2", (64, 256), FP16); di("ghI2", (64, 256), FP16)
    di("gw2e", (65, 256), FP16)
    di("mcR", (32, 2, 16, 32)); di("mcI", (32, 2, 16, 32))
    di("ident16", (64, 64), FP16)
    di("wlocT2", (128, 64), FP16)
    di("wm1T2", (128, 128), FP16)
    di("wm2T", (128, 32), FP16)
    di("ktp8", (128, len(TAP_PAIRS), 2, 128), FP8)
    di("bc_row", (1, 8192), FP16)
    di("bm1", (128, 1))
    di("gam", (32, 1)); di("bet", (32, 1))
    di("qones", (128, 32)); di("qonesT", (32, 128))
    outp = nc.declare_dram_parameter("outp", [128, FQ], FP16, isOutput=True)

    with tile.TileContext(nc) as tc, ExitStack() as ctx:
        _body(ctx, tc, P, outp)
    nc.finalize()
    return nc


def _body(ctx, tc, P, outp):
    nc = tc.nc
    xbh_f = P["xbh"].rearrange("c h w -> c (h w)")       # (64, 65536) fp16

    consts = ctx.enter_context(tc.tile_pool(name="consts", bufs=1))

    def load_const(name, shape, dtype=F32):
        t = consts.tile(list(shape), dtype, tag=name)
        nc.sync.dma_start(out=t, in_=P[name][:])
        return t

    main = ctx.enter_context(tc.tile_pool(name="main", bufs=1))
    mid = ctx.enter_context(tc.tile_pool(name="mid", bufs=1))

    # ---------------- S1 + S2: spectral (pool also scopes init DMAs) ----
    fhT_s = consts.tile([128, 2, 64], FP16, tag="fhT")
    nc.sync.dma_start(out=fhT_s, in_=P["fhT"].rearrange("(t p) m -> p t m", p=128))

    with tc.tile_pool(name="spec1", bufs=1) as sp1, \
         tc.tile_pool(name="xhp", bufs=4) as xhp:
        xhbs = []
        for blk in range(4):
            xhb = xhp.tile([128, 2, 2048], FP16, tag="xhb")
            for ht in range(2):
                nc.sync.dma_start(
                    out=xhb[:, ht, :],
                    in_=P["xhT"][ht, :, blk * 2048:(blk + 1) * 2048])
            xhbs.append(xhb)

        fwA_s = consts.tile([128, 2, 64], FP16, tag="fwA")
        nc.sync.dma_start(out=fwA_s, in_=P["fwA"].rearrange("(t p) m -> p t m", p=128))
        fwB_s = consts.tile([128, 2, 64], FP16, tag="fwB")
        nc.sync.dma_start(out=fwB_s, in_=P["fwB"].rearrange("(t p) m -> p t m", p=128))
        ident_s = load_const("ident16", (64, 64), FP16)
        ghR2_s = load_const("ghR2", (64, 256), FP16)
        ghI2_s = load_const("ghI2", (64, 256), FP16)
        gw2e_s = load_const("gw2e", (65, 256), FP16)

        xq8 = main.tile([128, XB_P], FP8, tag="xq8")
        for s in range(2):
            c0 = s * (XB_P // 2)
            c1 = XB_P if s else (XB_P // 2)
            nc.sync.dma_start(out=xq8[:, c0:c1], in_=P["xq8"][:, c0:c1])
        ktp8_s = consts.tile([128, len(TAP_PAIRS), 2, 128], FP8, tag="ktp8")
        nc.sync.dma_start(out=ktp8_s, in_=P["ktp8"][:])
        wlocT2_s = load_const("wlocT2", (128, 64), FP16)
        wm1T2_s = load_const("wm1T2", (128, 128), FP16)
        wm2T_s = load_const("wm2T", (128, 32), FP16)
        mcR_s = load_const("mcR", (32, 2, 16, 32))
        mcI_s = load_const("mcI", (32, 2, 16, 32))
        bm1_s = load_const("bm1", (128, 1))
        gam_s = load_const("gam", (32, 1))
        bet_s = load_const("bet", (32, 1))
        qones_s = load_const("qones", (128, 32))
        qonesT_s = load_const("qonesT", (32, 128))

        QstA = mid.tile([64, 8, 2, 32], FP16, tag="QstA")
        QstB = mid.tile([64, 8, 2, 32], FP16, tag="QstB")
        Qsth = [QstA, QstB]
        Qst_ch = [QstA.rearrange("p a b w -> p (a b) w"),
                  QstB.rearrange("p a b w -> p (a b) w")]        # (64, 16, 32)
        Zh2e = mid.tile([65, 4, 32, 64], FP16, tag="Zh2e")
        nc.sync.dma_start(out=Zh2e[64:65, :, :, :],
                          in_=P["bc_row"].rearrange("p (q c l) -> p q c l",
                                                    q=4, c=32))

        zbuf = main.tile([128, FQ], FP16, tag="zbuf")
        szc = main.tile([128, NHC], F32, tag="szc")
        sqc = main.tile([128, NHC], F32, tag="sqc")

        T1 = sp1.tile([64, OC, 256], FP16, tag="T1")             # (rmRI, c, w)
        T1v = T1.rearrange("p c w -> p (c w)")
        # PE warm-up while DMAs land (output read once, then overwritten)
        with tc.tile_pool(name="ps_warm", bufs=1, space="PSUM") as ps_w:
            wps = ps_w.tile([64, 128], F32, tag="warm")
            fhflat = fhT_s.rearrange("p t m -> p (t m)")
            for wi in range(40):
                nc.tensor.matmul(out=wps, lhsT=fhT_s[:, 0, :], rhs=fhflat,
                                 start=True, stop=True)
            nc.scalar.copy(out=T1v[:, 0:128], in_=wps)
        T1T0 = sp1.tile([128, 2, OC, 32], FP16, tag="T1T0")
        T1T1 = sp1.tile([128, 2, OC, 32], FP16, tag="T1T1")
        T1T = [T1T0, T1T1]

        with tc.tile_pool(name="ps_t1", bufs=2, space="PSUM") as ps_t1, \
             tc.tile_pool(name="ps_tr", bufs=2, space="PSUM") as ps_tr:
            for blk in range(4):
                xhb = xhbs[blk]
                for sub in range(2):
                    reg = blk * 2 + sub
                    pt = ps_t1.tile([64, 1024], F32, tag="t1p")
                    for _w in range(6):
                        nc.tensor.matmul(out=pt[:, 0:64],
                                         lhsT=fhT_s[:, 0, :],
                                         rhs=fhT_s[:, 0, :],
                                         start=True, stop=True)
                    for n in range(2):
                        col = sub * 1024 + n * 512
                        for ht in range(2):
                            nc.tensor.matmul(
                                out=pt[:, n * 512:(n + 1) * 512],
                                lhsT=fhT_s[:, ht, :],
                                rhs=xhb[:, ht, col:col + 512],
                                start=(ht == 0), stop=(ht == 1))
                    nc.scalar.copy(out=T1v[:, reg * 1024:(reg + 1) * 1024],
                                   in_=pt)

            for wh in range(2):
                for cb in range(4):
                    pt2 = ps_tr.tile([128, 512], FP16, tag="trp")
                    for i in range(8):
                        cch = cb * 8 + i
                        nc.tensor.transpose(
                            out=pt2[:, i * 64:(i + 1) * 64],
                            in_=T1[:, cch, wh * 128:(wh + 1) * 128],
                            identity=ident_s)
                    # one fused strided copy per (wh, cb) on the DVE --
                    # keeps the Act free so part1(0/1) gelus start early
                    ptv2 = pt2.rearrange("p (c a b) -> p a c b", c=8, a=2)
                    nc.vector.tensor_copy(
                        out=T1T[wh][:, :, cb * 8:(cb + 1) * 8, :],
                        in_=ptv2)

        # ---- S2 (cp/Qst/Zh) shares psum pools with the loop; part1(0/1)
        # interleave into the Zh stage to keep the PE warm through the
        # spectral tail and hide Qst/copy latencies.
        with tc.tile_pool(name="xqp", bufs=4) as xqp, \
             tc.tile_pool(name="h1sp", bufs=8) as h1sp, \
             tc.tile_pool(name="sqscr", bufs=2) as sqscr, \
             tc.tile_pool(name="ps_zp", bufs=2, space="PSUM") as ps_zp, \
             tc.tile_pool(name="ps_h1", bufs=2, space="PSUM") as ps_h1:
            ZPs = [None] * NHC
            h1ss = [None] * NHC
            xqt = [None] * (NHC // 2)

            def tap_rhs(pr, base):
                d0, d1 = TAP_PAIRS[pr]
                return APc(xq8.tensor, xq8.offset + base + d0,
                           [list(xq8.ap[0]), [d1 - d0, 2], [WP, 2], [1, 256]])

            def mlp1(hc, q):
                """mlp1 quarter q into a fresh psum buffer + immediate gelu."""
                xq2 = xqt[hc // 2]
                xoff = (hc % 2) * HC
                j = q % 2
                hp = ps_h1.tile([128, HC], F32, tag="h1p")
                for s2 in range(0, HC, 512):
                    nc.tensor.matmul(
                        out=hp[:, s2:s2 + 512],
                        lhsT=wm1T2_s[64 * j:64 * j + 64, :],
                        rhs=xq2[q // 2][64 * j:64 * j + 64,
                                        xoff + s2:xoff + s2 + 512],
                        start=True, stop=True, tile_position=(64 * j, 0))
                h1s = h1sp.tile([128, HC], FP16, tag="h1s")
                nc.scalar.activation(out=h1s, in_=hp, func=AF.Gelu,
                                     bias=bm1_s, scale=1.0)
                h1ss[hc].append(h1s)

            def tap(hc, t8):
                rg, pr = t8 // 4, t8 % 4
                ZPv = ZPs[hc].rearrange("p (g a b) -> p g a b", g=2, a=2)
                base = HALO_P + WP * (4 * hc + 2 * rg)
                nc.tensor.matmul(
                    out=ZPv[:, rg, :, :], lhsT=ktp8_s[:, pr, :, :],
                    rhs=tap_rhs(pr, base),
                    start=False, stop=False, skip_group_check=True,
                    perf_mode=mybir.MatmulPerfMode.DoubleRow)

            def part1(hc):
                f0 = hc * HC
                ZP = ps_zp.tile([128, HC], F32, tag="ZP")
                ZPs[hc] = ZP
                h1ss[hc] = []
                if hc % 2 == 0:
                    xq = []
                    for qp in range(2):
                        t = xqp.tile([128, 2 * HC], FP16, tag="xq")
                        for j in range(2):
                            q = qp * 2 + j
                            nc.sync.dma_start(
                                out=t[64 * j:64 * j + 64, :],
                                in_=xbh_f[:, q * FQ + f0:q * FQ + f0 + 2 * HC])
                        xq.append(t)
                    xqt[hc // 2] = xq
                xq2 = xqt[hc // 2]
                xoff = (hc % 2) * HC

                mlp1(hc, 0)
                mlp1(hc, 1)
                # local (q-paired block-diag): start=True
                for qp in range(2):
                    tp = (0, 64 * qp) if qp > 0 else None
                    for s in range(0, HC, 512):
                        nc.tensor.matmul(
                            out=ZP[64 * qp:64 * qp + 64, s:s + 512],
                            lhsT=wlocT2_s,
                            rhs=xq2[qp][:, xoff + s:xoff + s + 512],
                            start=True, stop=False, skip_group_check=True,
                            tile_position=tp)
                for t5 in range(5):
                    tap(hc, t5)

            def part2a(hc):
                mlp1(hc, 2)
                mlp1(hc, 3)
                for t8 in range(5, 8):
                    tap(hc, t8)

            def part2b(hc):
                ZP = ZPs[hc]
                for i in range(4):
                    h0 = hc * 4 + i
                    nc.tensor.matmul(out=ZP[:, i * 256:(i + 1) * 256],
                                     lhsT=Zh2e[:, :, :, h0], rhs=gw2e_s,
                                     start=False, stop=False,
                                     skip_group_check=True)

            def part2(hc):
                part2a(hc)
                part2b(hc)

            def finish(hc):
                f0 = hc * HC
                ZP = ZPs[hc]
                for q in range(NQ):
                    tp = (0, 32 * q) if q > 0 else None
                    for s in range(0, HC, 512):
                        nc.tensor.matmul(out=ZP[32 * q:32 * q + 32, s:s + 512],
                                         lhsT=wm2T_s,
                                         rhs=h1ss[hc][q][:, s:s + 512],
                                         start=False, stop=True,
                                         tile_position=tp,
                                         skip_group_check=True)
                # zbuf = ZP (fp16), accumulate sum into szc (DVE)
                nc.vector.tensor_scalar(
                    out=zbuf[:, f0:f0 + HC], in0=ZP, scalar1=0.0, scalar2=0.0,
                    op0=AX.add, op1=AX.add, accum_out=szc[:, hc:hc + 1])
                # square + sum; last hc on DVE to shorten the tail chain
                scr = sqscr.tile([128, HC], FP16, tag="scr")
                if hc == NHC - 1:
                    nc.vector.tensor_tensor(out=scr, in0=zbuf[:, f0:f0 + HC],
                                            in1=zbuf[:, f0:f0 + HC],
                                            op=AX.mult)
                else:
                    nc.gpsimd.tensor_tensor(out=scr, in0=zbuf[:, f0:f0 + HC],
                                            in1=zbuf[:, f0:f0 + HC],
                                            op=AX.mult)
                nc.vector.tensor_reduce(out=sqc[:, hc:hc + 1], in_=scr,
                                        axis=mybir.AxisListType.X, op=AX.add)

            # ---- cp stage: psum views carved from full-size ps_h1 tiles ----
            cpt = [ps_h1.tile([128, 1024], F32, tag="h1p",
                              name=f"cpt{_i}") for _i in range(2)]
            cph = [t[0:64, 0:512].rearrange("p (a b c) -> p a b c", a=8, b=2)
                   for t in cpt]
            tmpA = sp1.tile([32, 8, 32], F32, tag="mtmpA")
            tmpB = sp1.tile([32, 8, 32], F32, tag="mtmpB")
            for half in range(2):
                cp = cph[half]
                for pr in range(8 * half, 8 * half + 8):
                    dst = cp[:, pr - 8 * half, :, :].rearrange(
                        "p a b -> p (a b)")
                    for wh in range(2):
                        nc.tensor.matmul(out=dst,
                                         lhsT=T1T[wh][:, 0, 2 * pr:2 * pr + 2, :],
                                         rhs=fwA_s[:, wh, :],
                                         start=(wh == 0), stop=False)
                    for wh in range(2):
                        nc.tensor.matmul(out=dst,
                                         lhsT=T1T[wh][:, 1, 2 * pr:2 * pr + 2, :],
                                         rhs=fwB_s[:, wh, :],
                                         start=False, stop=(wh == 1))
                prs = slice(8 * half, 8 * half + 8)
                Qh = Qsth[half]
                for par in range(2):
                    crs = cp[32 * par:32 * par + 32, :, 0, :]
                    cis = cp[32 * par:32 * par + 32, :, 1, :]
                    mr = mcR_s[:, par, prs, :]
                    mi = mcI_s[:, par, prs, :]
                    nc.vector.tensor_tensor(out=tmpA, in0=crs, in1=mr, op=AX.mult)
                    nc.vector.tensor_tensor(out=tmpB, in0=cis, in1=mi, op=AX.mult)
                    nc.vector.tensor_tensor(out=Qh[0:32, :, par, :],
                                            in0=tmpA, in1=tmpB, op=AX.subtract)
                    nc.vector.tensor_tensor(out=tmpA, in0=cis, in1=mr, op=AX.mult)
                    nc.vector.tensor_tensor(out=tmpB, in0=crs, in1=mi, op=AX.mult)
                    nc.vector.tensor_tensor(out=Qh[32:64, :, par, :],
                                            in0=tmpA, in1=tmpB, op=AX.add)

            def zh(grp):
                t = ps_zp.tile([128, 1024], F32, tag="ZP", name=f"zh{grp}")
                zp = t[0:64, :].rearrange("p (c l) -> p c l", c=4)
                for i in range(4):
                    cch = grp * 4 + i
                    lhs = Qst_ch[grp // 4][:, cch - 16 * (grp // 4), :]
                    nc.tensor.matmul(out=zp[0:32, i, :], lhsT=lhs, rhs=ghR2_s,
                                     start=True, stop=True)
                    nc.tensor.matmul(out=zp[32:64, i, :], lhsT=lhs, rhs=ghI2_s,
                                     start=True, stop=True,
                                     tile_position=(0, 32))
                nc.scalar.copy(
                    out=Zh2e[0:64, :, grp * 4:(grp + 1) * 4, :],
                    in_=zp.rearrange("p c (q l) -> p q c l", q=4))

            for g in range(8):
                zh(g)
            part1(0)
            part1(1)

            for hc in range(NHC + 1):
                if 2 <= hc < NHC:
                    part1(hc)
                if hc >= 1:
                    finish(hc - 1)
                if hc < NHC:
                    part2(hc)

    # ---------------- S5: stats ----------------
    st = ctx.enter_context(tc.tile_pool(name="stats", bufs=1))
    with tc.tile_pool(name="ps_st", bufs=1, space="PSUM") as ps_st:
        sums = st.tile([128, 2], F32, tag="sums")
        nc.vector.tensor_reduce(out=sums[:, 0:1], in_=szc,
                                axis=mybir.AxisListType.X, op=AX.add)
        nc.vector.tensor_reduce(out=sums[:, 1:2], in_=sqc,
                                axis=mybir.AxisListType.X, op=AX.add)
        sp = ps_st.tile([32, 2], F32, tag="sp")
        nc.tensor.matmul(out=sp, lhsT=qones_s, rhs=sums, start=True, stop=True)
        mu = st.tile([32, 1], F32, tag="mu")
        negmu = st.tile([32, 1], F32, tag="negmu")
        ex2 = st.tile([32, 1], F32, tag="ex2")
        var = st.tile([32, 1], F32, tag="var")
        s12 = st.tile([32, 2], F32, tag="s12")
        inv_n = 1.0 / float(HW)
        nc.vector.tensor_scalar(out=mu, in0=sp[:, 0:1], scalar1=inv_n,
                                scalar2=None, op0=AX.mult)
        nc.vector.tensor_scalar(out=negmu, in0=sp[:, 0:1], scalar1=-inv_n,
                                scalar2=None, op0=AX.mult)
        nc.vector.tensor_scalar(out=ex2, in0=sp[:, 1:2], scalar1=inv_n,
                                scalar2=None, op0=AX.mult)
        nc.vector.scalar_tensor_tensor(out=var, in0=mu, scalar=negmu, in1=ex2,
                                       op0=AX.mult, op1=AX.add)
        epst = st.tile([32, 1], F32, tag="epst")
        nc.vector.memset(epst, 1e-5)
        nc.scalar.activation(out=var, in_=var, func=AF.Sqrt, bias=epst, scale=1.0)
        nc.vector.reciprocal(out=var, in_=var)                   # rstd
        nc.vector.tensor_tensor(out=s12[:, 0:1], in0=var, in1=gam_s, op=AX.mult)
        nc.vector.tensor_scalar(out=negmu, in0=mu, scalar1=-1.0,
                                scalar2=None, op0=AX.mult)
        nc.vector.scalar_tensor_tensor(out=s12[:, 1:2], in0=s12[:, 0:1],
                                       scalar=negmu, in1=bet_s,
                                       op0=AX.mult, op1=AX.add)
        spb = ps_st.tile([128, 2], F32, tag="spb")
        nc.tensor.matmul(out=spb, lhsT=qonesT_s, rhs=s12, start=True, stop=True)
        s12s = st.tile([128, 2], F32, tag="s12s")
        nc.vector.tensor_copy(out=s12s, in_=spb)

    # ---------------- S6: sweep 2 (gelu only; +x residual on host) ----
    with tc.tile_pool(name="sw2g", bufs=3) as sw2g:
        for ch in range(4):
            f0 = ch * 4096
            g = sw2g.tile([128, 4096], FP16, tag="g")
            nc.scalar.activation(out=g, in_=zbuf[:, f0:f0 + 4096], func=AF.Gelu,
                                 bias=s12s[:, 1:2], scale=s12s[:, 0:1])
            for s in range(0, 4096, 2048):
                nc.sync.dma_start(out=outp[:, f0 + s:f0 + s + 2048],
                                  in_=g[:, s:s + 2048])


_PROGRAM = None


def kernel(**inputs):
    global _PROGRAM
    in_maps = _per_core_inputs(inputs)
    if _PROGRAM is None:
        _PROGRAM = _build_program()
    res = run_bass_kernel_spmd(_PROGRAM, in_maps, list(range(N_CORES)))
    x = np.asarray(inputs["x"], np.float32)
    return _assemble(x, res.results)
